# revision 1
# baseline (speedup 1.0000x reference)
"""BERTgrid generator kernel for Trainium2 (8 NeuronCores, batch-parallel).

Per core (one document):
  emb [512, 768] f32, coors [512, 4] i32, mask [512, 1] i32
  -> out [768, 128*96] f32   (channel-major grid)

Device algorithm (no host compute on input values):
  1. valid/new_word/seg via triangular-matmul cumsums.
  2. Word mean table (shifted by one word) via one-hot matmul + reciprocal.
  3. Per-pixel last-covering-word index via two exponent-weighted matmuls:
     S1 = sum_words 128^(seg//32) over covering boxes  -> max chunk via f32
     exponent field; M_k = sum_words 4^(seg%32) per chunk -> max offset.
     All index math is exact (integer ops on the exponent bits).
  4. Paint: out[d, p] = table[widx[p], d] as table^T @ onehot(widx) in fp16
     (one-hot has a single 1 per column, so fp16 only rounds table values).
"""

import sys

import numpy as np

try:
    import concourse.bass as bass
except ImportError:  # grading env fallback
    sys.path.insert(0, "/opt/trn_rl_repo")
    import concourse.bass as bass

from concourse import bacc
import concourse.tile as tile
from concourse import mybir
from concourse.bass_utils import run_bass_kernel_spmd
from contextlib import ExitStack

P = 128
S, D = 512, 768
R, C, STRIDE = 128, 96, 8
T = S // P            # token tiles
WT = S // P           # word tiles / k-chunks
NPIX = R * C          # 12288
PG = 2048             # pixels per paint group
NG = NPIX // PG
DT = D // P

F32 = mybir.dt.float32
F16 = mybir.dt.float16
BF16 = mybir.dt.bfloat16
I32 = mybir.dt.int32
OP = mybir.AluOpType

_last_results = None


def _build():
    nc = bacc.Bacc(None, target_bir_lowering=False)
    emb_ext = nc.declare_dram_parameter("emb", [S, D], F32, isOutput=False)
    coors_ext = nc.declare_dram_parameter("coors", [S, 4], I32, isOutput=False)
    mask_ext = nc.declare_dram_parameter("mask", [S, 1], I32, isOutput=False)
    out_ext = nc.declare_dram_parameter("out", [D, NPIX], F32, isOutput=True)
    widx_dram = nc.dram_tensor("widx_scratch", [P, C], F16)

    with tile.TileContext(nc) as tc, ExitStack() as ctx:
        sing = ctx.enter_context(tc.tile_pool(name="sing", bufs=1))

        # ---- constants ----
        def iota_tile(name, shape, pattern, base, cm, out_dt=F32):
            it = sing.tile(shape, I32, tag=name + "_i")
            nc.gpsimd.iota(it[:], pattern, base=base, channel_multiplier=cm)
            if out_dt == I32:
                return it
            ft = sing.tile(shape, out_dt, tag=name)
            nc.vector.tensor_copy(ft[:], it[:])
            return ft

        iota_r = iota_tile("iota_r", [P, R], [[1, R]], 0, 0)          # 0..127 along free
        iota_c = iota_tile("iota_c", [P, C], [[1, C]], 0, 0)          # 0..95
        iota16 = iota_tile("iota16", [P, 16], [[1, 16]], 0, 0)        # 0..15
        iotaW = iota_tile("iotaW", [P, S], [[1, S]], -1, 0)           # word-1 values
        iotawp = [iota_tile(f"iotawp{kc}", [P, 1], [[0, 1]], kc * P, 1)
                  for kc in range(WT)]                                 # per-partition word id

        chunk16_i = sing.tile([P, 16, C], I32, tag="chunk16_i")
        nc.gpsimd.iota(chunk16_i[:], [[1, 16], [0, C]], base=0, channel_multiplier=0)
        chunk16f = sing.tile([P, 16 * C], F32, tag="chunk16f")
        nc.vector.tensor_copy(chunk16f[:].rearrange("p (a b) -> p a b", a=16),
                              chunk16_i[:])

        tri_i = sing.tile([P, P], I32, tag="tri_i")
        nc.gpsimd.iota(tri_i[:], [[1, P]], base=0, channel_multiplier=-1)  # i - j
        tri_f = sing.tile([P, P], F32, tag="tri_f")
        nc.vector.tensor_copy(tri_f[:], tri_i[:])
        tri = sing.tile([P, P], BF16, tag="tri")                       # [j, i] = (j <= i)
        nc.vector.tensor_scalar(out=tri[:], in0=tri_f[:], scalar1=0.0,
                                scalar2=None, op0=OP.is_ge)
        ones_bf = sing.tile([P, P], BF16, tag="ones_bf")
        nc.vector.memset(ones_bf[:], 1.0)

        # ---- token-tile loads (batched: column t = token tile) ----
        mask_all = sing.tile([P, T], I32, tag="mask_all")
        coors_all = sing.tile([P, 4 * T], I32, tag="coors_all")
        coorsm1_all = sing.tile([P, 4 * T], I32, tag="coorsm1_all")
        nc.vector.memset(coorsm1_all[0:1, 0:4], -1)
        embext = []
        for t in range(T):
            nc.sync.dma_start(out=mask_all[:, t:t + 1],
                              in_=mask_ext[t * P:(t + 1) * P, :])
            nc.sync.dma_start(out=coors_all[:, 4 * t:4 * t + 4],
                              in_=coors_ext[t * P:(t + 1) * P, :])
            if t == 0:
                nc.sync.dma_start(out=coorsm1_all[1:P, 0:4],
                                  in_=coors_ext[0:P - 1, :])
            else:
                nc.sync.dma_start(out=coorsm1_all[:, 4 * t:4 * t + 4],
                                  in_=coors_ext[t * P - 1:(t + 1) * P - 1, :])
            et = sing.tile([P, D + 1], F16, tag=f"emb{t}")
            nc.vector.memset(et[:, D:D + 1], 1.0)
            nc.gpsimd.dma_start(out=et[:, 0:D], in_=emb_ext[t * P:(t + 1) * P, :])
            embext.append(et)

        # ---- batched per-token quantities ----
        mf = sing.tile([P, T], F32, tag="maskf")
        nc.vector.tensor_copy(mf[:], mask_all[:])
        invm4 = sing.tile([P, T], BF16, tag="invm4")
        nc.vector.tensor_scalar(out=invm4[:], in0=mf[:], scalar1=1.0,
                                scalar2=-1.0, op0=OP.subtract, op1=OP.mult)
        cf = sing.tile([P, 4 * T], F32, tag="coorsf")
        nc.vector.tensor_copy(cf[:], coors_all[:])
        cm1f = sing.tile([P, 4 * T], F32, tag="coorsm1f")
        nc.vector.tensor_copy(cm1f[:], coorsm1_all[:])
        eq16 = sing.tile([P, 4 * T], F32, tag="eq16")
        nc.vector.tensor_tensor(eq16[:], cf[:], cm1f[:], OP.is_equal)
        same4 = sing.tile([P, T], F32, tag="same4")
        nc.vector.tensor_reduce(same4[:],
                                eq16[:].rearrange("p (t c) -> p t c", t=T),
                                mybir.AxisListType.X, OP.min)
        wci = sing.tile([P, 4 * T], I32, tag="wci")
        nc.vector.tensor_scalar(out=wci[:], in0=coors_all[:], scalar1=3,
                                scalar2=None, op0=OP.arith_shift_right)
        wcf = sing.tile([P, 4 * T], F32, tag="wcf")
        nc.vector.tensor_copy(wcf[:], wci[:])

        # ---- valid = (cumsum(1-mask) == 0), seg = cumsum(new_word) - 1 ----
        valid4 = sing.tile([P, T], F32, tag="valid4")
        nw4 = sing.tile([P, T], F32, tag="nw4")
        nwb4 = sing.tile([P, T], BF16, tag="nwb4")
        seg4 = sing.tile([P, T], F32, tag="seg4")
        segi4 = sing.tile([P, T], I32, tag="segi4")
        with tc.tile_pool(name="psA", bufs=1, space="PSUM") as psA:
            for mt in range(T):
                vp = psA.tile([P, 1], F32, tag=f"v{mt}", name=f"v{mt}")
                for kc in range(mt + 1):
                    nc.tensor.matmul(out=vp[:],
                                     lhsT=(tri[:] if kc == mt else ones_bf[:]),
                                     rhs=invm4[:, kc:kc + 1],
                                     start=(kc == 0), stop=(kc == mt))
                nc.vector.tensor_scalar(out=valid4[:, mt:mt + 1], in0=vp[:],
                                        scalar1=0.5, scalar2=None, op0=OP.is_lt)
            nc.vector.scalar_tensor_tensor(out=nw4[:], in0=same4[:], scalar=0.5,
                                           in1=valid4[:], op0=OP.is_lt, op1=OP.mult)
            nc.vector.tensor_copy(nwb4[:], nw4[:])
            for mt in range(T):
                sp = psA.tile([P, 1], F32, tag=f"s{mt}", name=f"s{mt}")
                for kc in range(mt + 1):
                    nc.tensor.matmul(out=sp[:],
                                     lhsT=(tri[:] if kc == mt else ones_bf[:]),
                                     rhs=nwb4[:, kc:kc + 1],
                                     start=(kc == 0), stop=(kc == mt))
                nc.vector.tensor_scalar(out=seg4[:, mt:mt + 1], in0=sp[:],
                                        scalar1=1.0, scalar2=None, op0=OP.subtract)
            nc.vector.tensor_copy(segi4[:], seg4[:])

        # ---- per-token scan weights (batched) ----
        chunk4_i = sing.tile([P, T], I32, tag="chunk4_i")
        nc.vector.tensor_scalar(out=chunk4_i[:], in0=segi4[:], scalar1=5,
                                scalar2=None, op0=OP.arith_shift_right)
        chunk4_f = sing.tile([P, T], F32, tag="chunk4_f")
        nc.vector.tensor_copy(chunk4_f[:], chunk4_i[:])
        w1b = sing.tile([P, T], I32, tag="w1b")
        nc.vector.tensor_scalar(out=w1b[:], in0=chunk4_i[:], scalar1=7,
                                scalar2=127, op0=OP.mult, op1=OP.add)
        nc.vector.tensor_scalar(out=w1b[:], in0=w1b[:], scalar1=23,
                                scalar2=None, op0=OP.logical_shift_left)
        cw1 = sing.tile([P, T], F32, tag="cw1")
        nc.vector.tensor_tensor(cw1[:], w1b[:].bitcast(F32), nw4[:], OP.mult)
        w2b = sing.tile([P, T], I32, tag="w2b")
        nc.vector.tensor_scalar(out=w2b[:], in0=segi4[:], scalar1=31,
                                scalar2=None, op0=OP.bitwise_and)
        nc.vector.tensor_scalar(out=w2b[:], in0=w2b[:], scalar1=1,
                                scalar2=None, op0=OP.logical_shift_left)
        nc.vector.tensor_scalar(out=w2b[:], in0=w2b[:], scalar1=127,
                                scalar2=None, op0=OP.add)
        nc.vector.tensor_scalar(out=w2b[:], in0=w2b[:], scalar1=23,
                                scalar2=None, op0=OP.logical_shift_left)
        cw2 = sing.tile([P, T], F32, tag="cw2")
        nc.vector.tensor_tensor(cw2[:], w2b[:].bitcast(F32), nw4[:], OP.mult)

        rowcov, rhs1, rhs2 = [], [], []
        for t in range(T):
            y0, y1 = wcf[:, 4 * t + 1:4 * t + 2], wcf[:, 4 * t + 3:4 * t + 4]
            x0, x1 = wcf[:, 4 * t + 0:4 * t + 1], wcf[:, 4 * t + 2:4 * t + 3]
            tge = sing.tile([P, R], F32, tag="tge")
            nc.vector.tensor_scalar(out=tge[:], in0=iota_r[:], scalar1=y0,
                                    scalar2=None, op0=OP.is_ge)
            rc = sing.tile([P, R], BF16, tag=f"rowcov{t}")
            nc.vector.scalar_tensor_tensor(out=rc[:], in0=iota_r[:], scalar=y1,
                                           in1=tge[:], op0=OP.is_lt, op1=OP.mult)
            rowcov.append(rc)
            cge = sing.tile([P, C], F32, tag="cge")
            nc.vector.tensor_scalar(out=cge[:], in0=iota_c[:], scalar1=x0,
                                    scalar2=None, op0=OP.is_ge)
            ccv = sing.tile([P, C], BF16, tag=f"colcov{t}")
            nc.vector.scalar_tensor_tensor(out=ccv[:], in0=iota_c[:], scalar=x1,
                                           in1=cge[:], op0=OP.is_lt, op1=OP.mult)
            r1 = sing.tile([P, C], BF16, tag=f"rhs1{t}")
            nc.vector.tensor_scalar(out=r1[:], in0=ccv[:],
                                    scalar1=cw1[:, t:t + 1],
                                    scalar2=None, op0=OP.mult)
            rhs1.append(r1)
            tmp16 = sing.tile([P, 16], BF16, tag="tmp16")
            nc.vector.tensor_scalar(out=tmp16[:], in0=iota16[:],
                                    scalar1=chunk4_f[:, t:t + 1],
                                    scalar2=cw2[:, t:t + 1],
                                    op0=OP.is_equal, op1=OP.mult)
            r2 = sing.tile([P, 16 * C], BF16, tag=f"rhs2{t}")
            nc.vector.tensor_tensor(
                r2[:].rearrange("p (a b) -> p a b", a=16),
                tmp16[:].unsqueeze(2).broadcast_to([P, 16, C]),
                ccv[:].unsqueeze(1).broadcast_to([P, 16, C]),
                OP.mult)
            rhs2.append(r2)

        # ---- index map via stage matmuls ----
        widx16 = sing.tile([P, C], F16, tag="widx16")
        widx_i = sing.tile([P, C], I32, tag="widx_i")
        with tc.tile_pool(name="psC", bufs=1, space="PSUM") as psC:
            ps1 = psC.tile([P, C], F32, tag="ps1")
            for kc in range(T):
                nc.tensor.matmul(out=ps1[:], lhsT=rowcov[kc][:], rhs=rhs1[kc][:],
                                 start=(kc == 0), stop=(kc == T - 1))
            ps2 = psC.tile([P, 16 * C], F32, tag="ps2")
            for n3 in range(3):
                sl = slice(n3 * 512, (n3 + 1) * 512)
                for kc in range(T):
                    nc.tensor.matmul(out=ps2[:, sl], lhsT=rowcov[kc][:],
                                     rhs=rhs2[kc][:, sl],
                                     start=(kc == 0), stop=(kc == T - 1))

            s1m = sing.tile([P, C], F32, tag="s1m")
            nc.vector.tensor_scalar(out=s1m[:], in0=ps1[:], scalar1=1.0,
                                    scalar2=None, op0=OP.max)
            e1 = sing.tile([P, C], I32, tag="e1")
            nc.vector.tensor_scalar(out=e1[:], in0=s1m[:].bitcast(I32), scalar1=23,
                                    scalar2=None, op0=OP.logical_shift_right)
            nc.vector.tensor_scalar(out=e1[:], in0=e1[:], scalar1=127,
                                    scalar2=None, op0=OP.subtract)
            cst_i = sing.tile([P, C], I32, tag="cst_i")
            nc.vector.tensor_scalar(out=cst_i[:], in0=e1[:], scalar1=9363,
                                    scalar2=None, op0=OP.mult)
            nc.vector.tensor_scalar(out=cst_i[:], in0=cst_i[:], scalar1=16,
                                    scalar2=None, op0=OP.arith_shift_right)
            cst_f = sing.tile([P, C], F32, tag="cst_f")
            nc.vector.tensor_copy(cst_f[:], cst_i[:])

            # msel[r, c] = ps2[r, cstar, c] via one-hot mask + k-reduce
            cmp16 = sing.tile([P, 16 * C], F32, tag="cmp16")
            nc.vector.tensor_tensor(
                cmp16[:].rearrange("p (a b) -> p a b", a=16),
                chunk16f[:].rearrange("p (a b) -> p a b", a=16),
                cst_f[:].unsqueeze(1).broadcast_to([P, 16, C]),
                OP.is_equal)
            nc.vector.tensor_tensor(cmp16[:], cmp16[:], ps2[:], OP.mult)
            msel = sing.tile([P, C], F32, tag="msel")
            nc.vector.tensor_reduce(msel[:],
                                    cmp16[:].rearrange("p (a b) -> p b a", a=16),
                                    mybir.AxisListType.X, OP.add)

            mm = sing.tile([P, C], F32, tag="mm")
            nc.vector.tensor_scalar(out=mm[:], in0=msel[:], scalar1=1.0,
                                    scalar2=None, op0=OP.max)
            e2 = sing.tile([P, C], I32, tag="e2")
            nc.vector.tensor_scalar(out=e2[:], in0=mm[:].bitcast(I32), scalar1=23,
                                    scalar2=None, op0=OP.logical_shift_right)
            nc.vector.tensor_scalar(out=e2[:], in0=e2[:], scalar1=127,
                                    scalar2=None, op0=OP.subtract)
            lo = sing.tile([P, C], I32, tag="lo")
            nc.vector.tensor_scalar(out=lo[:], in0=e2[:], scalar1=1,
                                    scalar2=None, op0=OP.arith_shift_right)
            nc.vector.tensor_scalar(out=widx_i[:], in0=cst_i[:], scalar1=5,
                                    scalar2=None, op0=OP.logical_shift_left)
            nc.vector.tensor_tensor(widx_i[:], widx_i[:], lo[:], OP.add)
            nc.vector.tensor_copy(widx16[:], widx_i[:])

        # round-trip through DRAM to flatten + broadcast across partitions
        nc.sync.dma_start(out=widx_dram[:], in_=widx16[:])
        widx_bc = sing.tile([P, NPIX], F16, tag="widx_bc")
        widx_flat = widx_dram[:].rearrange("p c -> (p c)")
        for g in range(NG):
            nc.gpsimd.dma_start(
                out=widx_bc[:, g * PG:(g + 1) * PG],
                in_=widx_flat[g * PG:(g + 1) * PG].partition_broadcast(P))

        # ---- word mean table (shifted by one word) ----
        # O'[i, w] = valid[i] * (seg[i] == w - 1); table[w] = sum/cnt, row 0 = 0
        table16 = []
        Opr = []
        for t in range(T):
            o = sing.tile([P, S], F16, tag=f"op{t}")
            nc.vector.tensor_scalar(out=o[:], in0=iotaW[:],
                                    scalar1=seg4[:, t:t + 1],
                                    scalar2=valid4[:, t:t + 1],
                                    op0=OP.is_equal, op1=OP.mult)
            Opr.append(o)
        with tc.tile_pool(name="psD", bufs=2, space="PSUM") as psD:
            for wt in range(WT):
                ptab = psD.tile([P, 1024], F32, tag="ptab")
                for kc in range(T):
                    lhs = Opr[kc][:, wt * P:(wt + 1) * P]
                    nc.tensor.matmul(out=ptab[:, 0:512], lhsT=lhs,
                                     rhs=embext[kc][:, 0:512],
                                     start=(kc == 0), stop=(kc == T - 1))
                    nc.tensor.matmul(out=ptab[:, 512:D + 1], lhsT=lhs,
                                     rhs=embext[kc][:, 512:D + 1],
                                     start=(kc == 0), stop=(kc == T - 1))
                rec = sing.tile([P, 1], F32, tag="rec")
                nc.vector.tensor_scalar(out=rec[:], in0=ptab[:, D:D + 1],
                                        scalar1=1.0, scalar2=None, op0=OP.max)
                recr = sing.tile([P, 1], F32, tag="recr")
                nc.vector.reciprocal(recr[:], rec[:])
                tb = sing.tile([P, D], F16, tag=f"table{wt}")
                nc.vector.tensor_scalar(out=tb[:], in0=ptab[:, 0:D],
                                        scalar1=recr[:, 0:1], scalar2=None,
                                        op0=OP.mult)
                table16.append(tb)

        # ---- paint: out[d, p] = table[widx[p], d] ----
        NH = PG // 512  # matmul column-slices per psum tile
        with tc.tile_pool(name="oh", bufs=2) as ohp, \
             tc.tile_pool(name="stage", bufs=3) as stp, \
             tc.tile_pool(name="pp", bufs=2, space="PSUM") as ppp:

            def emit_paint(nkc):
                for g in range(NG):
                    gs = slice(g * PG, (g + 1) * PG)
                    ohs = []
                    for kc in range(nkc):
                        oh = ohp.tile([P, PG], F16, tag=f"oh{kc}", name=f"oh{kc}")
                        nc.vector.tensor_scalar(out=oh[:], in0=widx_bc[:, gs],
                                                scalar1=iotawp[kc][:, 0:1],
                                                scalar2=None, op0=OP.is_equal)
                        ohs.append(oh)
                    for dt in range(DT):
                        stage = stp.tile([P, PG], F32, tag="stage", name="stage")
                        pp = ppp.tile([P, PG], F32, tag="pp", name="pp")
                        dsl = slice(dt * P, (dt + 1) * P)
                        for kc in range(nkc):
                            for s3 in range(NH):
                                nc.tensor.matmul(
                                    out=pp[:, s3 * 512:(s3 + 1) * 512],
                                    lhsT=table16[kc][:, dsl],
                                    rhs=ohs[kc][:, s3 * 512:(s3 + 1) * 512],
                                    start=(kc == 0), stop=(kc == nkc - 1))
                        if dt % 2 == 0:
                            nc.scalar.copy(out=stage[:], in_=pp[:])
                        else:
                            nc.vector.tensor_copy(stage[:], pp[:])
                        nc.sync.dma_start(out=out_ext[dt * P:(dt + 1) * P, gs],
                                          in_=stage[:])

            emit_paint(WT)
    nc.compile()
    return nc


_nc_cache = None


def kernel(bert_embeddings, coors, mask, image_h=1024, image_w=768, stride=8):
    global _last_results, _nc_cache
    emb = np.ascontiguousarray(np.asarray(bert_embeddings, dtype=np.float32))
    co = np.ascontiguousarray(np.asarray(coors, dtype=np.int32))
    mk = np.ascontiguousarray(np.asarray(mask, dtype=np.int32))
    ih, iw, st = int(image_h), int(image_w), int(stride)
    B = emb.shape[0]
    assert (ih // st, iw // st) == (R, C) and st == STRIDE
    assert emb.shape == (B, S, D) and B == 8

    if _nc_cache is None:
        _nc_cache = _build()
    nc = _nc_cache

    in_maps = [{"emb": emb[b], "coors": co[b], "mask": mk[b].reshape(S, 1)}
               for b in range(B)]
    res = run_bass_kernel_spmd(nc, in_maps, core_ids=list(range(B)))
    _last_results = res
    out = np.stack([np.asarray(res.results[b]["out"]).reshape(D, R, C)
                    for b in range(B)])
    return out.astype(np.float32)



# revision 4
# speedup vs baseline: 1.3040x; 1.3040x over previous
"""BERTgrid generator kernel for Trainium2 (8 NeuronCores, batch-parallel).

Per core (one document):
  emb [512, 768] f32, coors [512, 4] i32, mask [512, 1] i32
  -> out [768, 128*96] f32   (channel-major grid)

Device algorithm (no host compute on input values):
  1. valid/new_word/seg via triangular-matmul cumsums.
  2. Word mean table (shifted by one word) via one-hot matmul + reciprocal.
     Words spans >=2 tokens, so word ids < 256 -> 2 chunks of 128.
  3. Per-pixel last-covering-word index via two exponent-weighted matmuls:
     S1 = sum_words 128^(seg//32) over covering boxes  -> max chunk via f32
     exponent field; M_k = sum_words 4^(seg%32) per chunk -> max offset.
     All index math is exact (integer ops on the exponent bits).
  4. Paint: out[d, p] = table[widx[p], d] as table^T @ onehot(widx) in fp16
     (one-hot has a single 1 per column, so fp16 only rounds table values).
"""

import sys

import numpy as np

try:
    import concourse.bass as bass
except ImportError:  # grading env fallback
    sys.path.insert(0, "/opt/trn_rl_repo")
    import concourse.bass as bass

from concourse import bacc
import concourse.tile as tile
from concourse import mybir
from concourse.bass_utils import run_bass_kernel_spmd
from contextlib import ExitStack

P = 128
S, D = 512, 768
R, C, STRIDE = 128, 96, 8
T = S // P            # token tiles
NW = 256              # max word ids (each word spans >=2 tokens)
WT = NW // P          # word chunks for table/paint (2)
NCH = NW // 32        # seg chunks for the stage-2 scan (8)
NPIX = R * C          # 12288
PG = 2048             # pixels per paint group
NG = NPIX // PG
DT = D // P
NH = PG // 512        # matmul column-slices per psum tile

F32 = mybir.dt.float32
F16 = mybir.dt.float16
BF16 = mybir.dt.bfloat16
I32 = mybir.dt.int32
OP = mybir.AluOpType

_last_results = None


def _const_blocks():
    """Host-precomputed constants, embedded in the NEFF (input-independent)."""
    p = np.arange(P, dtype=np.float32)[:, None]
    iota_r = np.broadcast_to(np.arange(R, dtype=np.float32), (P, R))
    iota_c = np.broadcast_to(np.arange(C, dtype=np.float32), (P, C))
    iota_w = np.broadcast_to(np.arange(NW, dtype=np.float32) - 1.0, (P, NW))
    iota_8 = np.broadcast_to(np.arange(NCH, dtype=np.float32), (P, NCH))
    iota_wp = p + np.array([0.0, P], dtype=np.float32)[None, :]  # [P, WT]
    cf32 = np.concatenate([iota_r, iota_c, iota_w, iota_8, iota_wp], axis=1)
    ii = np.arange(P)
    tri = (ii[:, None] <= ii[None, :]).astype(np.float16)  # [j, i] = j <= i
    ones = np.ones((P, P), dtype=np.float16)
    cf16 = np.concatenate([tri, ones], axis=1)
    return np.ascontiguousarray(cf32), np.ascontiguousarray(cf16)


def _build():
    nc = bacc.Bacc(None, target_bir_lowering=False)
    emb_ext = nc.declare_dram_parameter("emb", [S, D], F32, isOutput=False)
    coors_ext = nc.declare_dram_parameter("coors", [S, 4], I32, isOutput=False)
    mask_ext = nc.declare_dram_parameter("mask", [S, 1], I32, isOutput=False)
    out_ext = nc.declare_dram_parameter("out", [D, NPIX], F32, isOutput=True)
    widx_dram = nc.dram_tensor("widx_scratch", [P, C], F16)
    cf32_np, cf16_np = _const_blocks()
    cf32_ext = nc.inline_tensor(cf32_np, "cons_f32")
    cf16_ext = nc.inline_tensor(cf16_np, "cons_f16")

    with tile.TileContext(nc) as tc, ExitStack() as ctx:
        sing = ctx.enter_context(tc.tile_pool(name="sing", bufs=1))

        # ---- const loads (scalar HWDGE queue; sync is used by inputs) ----
        NC32 = R + C + NW + NCH + WT
        cf32 = sing.tile([P, NC32], F32, tag="cf32")
        nc.scalar.dma_start(out=cf32[:], in_=cf32_ext[:])
        cf16 = sing.tile([P, 2 * P], F16, tag="cf16")
        nc.scalar.dma_start(out=cf16[:], in_=cf16_ext[:])
        o_r, o_c = 0, R
        o_w, o_8, o_wp = R + C, R + C + NW, R + C + NW + NCH
        iota_r = cf32[:, o_r:o_r + R]
        iota_c = cf32[:, o_c:o_c + C]
        iotaW = cf32[:, o_w:o_w + NW]
        iota8f = cf32[:, o_8:o_8 + NCH]
        iotawp = cf32[:, o_wp:o_wp + WT]
        tri = cf16[:, 0:P]
        ones16 = cf16[:, P:2 * P]

        # ---- input loads (batched via rearranged APs) ----
        mask_all = sing.tile([P, T], I32, tag="mask_all")
        nc.sync.dma_start(
            out=mask_all[:].rearrange("p (t o) -> p t o", t=T),
            in_=mask_ext[:].rearrange("(t p) o -> p t o", t=T))
        coors_all = sing.tile([P, 4 * T], I32, tag="coors_all")
        coors_re = coors_ext[:].rearrange("(t p) c -> p t c", t=T)
        nc.sync.dma_start(
            out=coors_all[:].rearrange("p (t c) -> p t c", t=T),
            in_=coors_re)
        coorsm1_all = sing.tile([P, 4 * T], I32, tag="coorsm1_all")
        nc.vector.memset(coorsm1_all[0:1, 0:4], -1)
        nc.sync.dma_start(
            out=coorsm1_all[1:P, :].rearrange("p (t c) -> p t c", t=T),
            in_=coors_re[0:P - 1])
        nc.sync.dma_start(
            out=coorsm1_all[0:1, 4:4 * T].rearrange("p (t c) -> p t c", t=T - 1),
            in_=coors_re[P - 1:P, 0:T - 1])
        # emb as f16 with a trailing ones column per token tile (for counts)
        embT = sing.tile([P, T * (D + 1)], F16, tag="embT")
        embT3 = embT[:].rearrange("p (t e) -> p t e", e=D + 1)
        nc.vector.memset(embT3[:, :, D:D + 1], 1.0)
        nc.gpsimd.dma_start(
            out=embT3[:, :, 0:D],
            in_=emb_ext[:].rearrange("(t p) d -> p t d", t=T))
        embext = [embT[:, t * (D + 1):(t + 1) * (D + 1)] for t in range(T)]

        # ---- batched per-token quantities ----
        mf = sing.tile([P, T], F32, tag="maskf")
        nc.vector.tensor_copy(mf[:], mask_all[:])
        invm4 = sing.tile([P, T], F16, tag="invm4")
        nc.vector.tensor_scalar(out=invm4[:], in0=mf[:], scalar1=1.0,
                                scalar2=-1.0, op0=OP.subtract, op1=OP.mult)
        cf = sing.tile([P, 4 * T], F32, tag="coorsf")
        nc.vector.tensor_copy(cf[:], coors_all[:])
        cm1f = sing.tile([P, 4 * T], F32, tag="coorsm1f")
        nc.vector.tensor_copy(cm1f[:], coorsm1_all[:])
        eq16 = sing.tile([P, 4 * T], F32, tag="eq16")
        nc.vector.tensor_tensor(eq16[:], cf[:], cm1f[:], OP.is_equal)
        same4 = sing.tile([P, T], F32, tag="same4")
        nc.vector.tensor_reduce(same4[:],
                                eq16[:].rearrange("p (t c) -> p t c", t=T),
                                mybir.AxisListType.X, OP.min)
        wci = sing.tile([P, 4 * T], I32, tag="wci")
        nc.vector.tensor_scalar(out=wci[:], in0=coors_all[:], scalar1=3,
                                scalar2=None, op0=OP.arith_shift_right)
        wcf = sing.tile([P, 4 * T], F32, tag="wcf")
        nc.vector.tensor_copy(wcf[:], wci[:])

        # ---- valid = (cumsum(1-mask) == 0), seg = cumsum(new_word) - 1 ----
        valid4 = sing.tile([P, T], F32, tag="valid4")
        nw4 = sing.tile([P, T], F32, tag="nw4")
        nwb4 = sing.tile([P, T], F16, tag="nwb4")
        seg4 = sing.tile([P, T], F32, tag="seg4")
        segi4 = sing.tile([P, T], I32, tag="segi4")
        with tc.tile_pool(name="psA", bufs=1, space="PSUM") as psA:
            vps = psA.tile([P, T], F32, tag="vps", name="vps")
            for mt in range(T):
                for kc in range(mt + 1):
                    nc.tensor.matmul(out=vps[:, mt:mt + 1],
                                     lhsT=(tri if kc == mt else ones16),
                                     rhs=invm4[:, kc:kc + 1],
                                     start=(kc == 0), stop=(kc == mt))
            nc.vector.tensor_scalar(out=valid4[:], in0=vps[:],
                                    scalar1=0.5, scalar2=None, op0=OP.is_lt)
            nc.vector.scalar_tensor_tensor(out=nw4[:], in0=same4[:], scalar=0.5,
                                           in1=valid4[:], op0=OP.is_lt,
                                           op1=OP.mult)
            nc.vector.tensor_copy(nwb4[:], nw4[:])
            sps = psA.tile([P, T], F32, tag="sps", name="sps")
            for mt in range(T):
                for kc in range(mt + 1):
                    nc.tensor.matmul(out=sps[:, mt:mt + 1],
                                     lhsT=(tri if kc == mt else ones16),
                                     rhs=nwb4[:, kc:kc + 1],
                                     start=(kc == 0), stop=(kc == mt))
            nc.vector.tensor_scalar(out=seg4[:], in0=sps[:],
                                    scalar1=1.0, scalar2=None, op0=OP.subtract)
            nc.vector.tensor_copy(segi4[:], seg4[:])

        # ---- per-token scan weights (batched) ----
        chunk4_i = sing.tile([P, T], I32, tag="chunk4_i")
        nc.vector.tensor_scalar(out=chunk4_i[:], in0=segi4[:], scalar1=5,
                                scalar2=None, op0=OP.arith_shift_right)
        chunk4_f = sing.tile([P, T], F32, tag="chunk4_f")
        nc.vector.tensor_copy(chunk4_f[:], chunk4_i[:])
        w1b = sing.tile([P, T], I32, tag="w1b")
        nc.vector.tensor_scalar(out=w1b[:], in0=chunk4_i[:], scalar1=7,
                                scalar2=127, op0=OP.mult, op1=OP.add)
        nc.vector.tensor_scalar(out=w1b[:], in0=w1b[:], scalar1=23,
                                scalar2=None, op0=OP.logical_shift_left)
        cw1 = sing.tile([P, T], F32, tag="cw1")
        nc.vector.tensor_tensor(cw1[:], w1b[:].bitcast(F32), nw4[:], OP.mult)
        w2b = sing.tile([P, T], I32, tag="w2b")
        nc.vector.tensor_scalar(out=w2b[:], in0=segi4[:], scalar1=31,
                                scalar2=None, op0=OP.bitwise_and)
        nc.vector.tensor_scalar(out=w2b[:], in0=w2b[:], scalar1=1,
                                scalar2=None, op0=OP.logical_shift_left)
        nc.vector.tensor_scalar(out=w2b[:], in0=w2b[:], scalar1=127,
                                scalar2=None, op0=OP.add)
        nc.vector.tensor_scalar(out=w2b[:], in0=w2b[:], scalar1=23,
                                scalar2=None, op0=OP.logical_shift_left)
        cw2 = sing.tile([P, T], F32, tag="cw2")
        nc.vector.tensor_tensor(cw2[:], w2b[:].bitcast(F32), nw4[:], OP.mult)

        # ---- per token-tile coverage + weighted rhs ----
        rowcov, rhs1, rhs2 = [], [], []
        for t in range(T):
            y0, y1 = wcf[:, 4 * t + 1:4 * t + 2], wcf[:, 4 * t + 3:4 * t + 4]
            x0, x1 = wcf[:, 4 * t + 0:4 * t + 1], wcf[:, 4 * t + 2:4 * t + 3]
            tge = sing.tile([P, R], F32, tag="tge")
            nc.vector.tensor_scalar(out=tge[:], in0=iota_r, scalar1=y0,
                                    scalar2=None, op0=OP.is_ge)
            rc = sing.tile([P, R], BF16, tag=f"rowcov{t}")
            nc.vector.scalar_tensor_tensor(out=rc[:], in0=iota_r, scalar=y1,
                                           in1=tge[:], op0=OP.is_lt,
                                           op1=OP.mult)
            rowcov.append(rc)
            cge = sing.tile([P, C], F32, tag="cge")
            nc.vector.tensor_scalar(out=cge[:], in0=iota_c, scalar1=x0,
                                    scalar2=None, op0=OP.is_ge)
            ccv = sing.tile([P, C], BF16, tag=f"colcov{t}")
            nc.vector.scalar_tensor_tensor(out=ccv[:], in0=iota_c, scalar=x1,
                                           in1=cge[:], op0=OP.is_lt,
                                           op1=OP.mult)
            r1 = sing.tile([P, C], BF16, tag=f"rhs1{t}")
            nc.vector.tensor_scalar(out=r1[:], in0=ccv[:],
                                    scalar1=cw1[:, t:t + 1],
                                    scalar2=None, op0=OP.mult)
            rhs1.append(r1)
            tmp8 = sing.tile([P, NCH], BF16, tag="tmp8")
            nc.vector.tensor_scalar(out=tmp8[:], in0=iota8f,
                                    scalar1=chunk4_f[:, t:t + 1],
                                    scalar2=cw2[:, t:t + 1],
                                    op0=OP.is_equal, op1=OP.mult)
            r2 = sing.tile([P, NCH * C], BF16, tag=f"rhs2{t}")
            nc.vector.tensor_tensor(
                r2[:].rearrange("p (a b) -> p a b", a=NCH),
                tmp8[:].unsqueeze(2).broadcast_to([P, NCH, C]),
                ccv[:].unsqueeze(1).broadcast_to([P, NCH, C]),
                OP.mult)
            rhs2.append(r2)

        # ---- word mean table (shifted by one word) on the PE in parallel ----
        # O'[i, w] = valid[i] * (seg[i] == w - 1); table[w] = sum/cnt, row 0 = 0
        Opr = []
        for t in range(T):
            o = sing.tile([P, NW], F16, tag=f"op{t}")
            nc.vector.tensor_scalar(out=o[:], in0=iotaW,
                                    scalar1=seg4[:, t:t + 1],
                                    scalar2=valid4[:, t:t + 1],
                                    op0=OP.is_equal, op1=OP.mult)
            Opr.append(o)

        # ---- index map via stage matmuls ----
        widx16 = sing.tile([P, C], F16, tag="widx16")
        widx_i = sing.tile([P, C], I32, tag="widx_i")
        table16 = []
        with tc.tile_pool(name="psC", bufs=1, space="PSUM") as psC, \
             tc.tile_pool(name="psD", bufs=2, space="PSUM") as psD:
            ps1 = psC.tile([P, C], F32, tag="ps1")
            for kc in range(T):
                nc.tensor.matmul(out=ps1[:], lhsT=rowcov[kc][:], rhs=rhs1[kc][:],
                                 start=(kc == 0), stop=(kc == T - 1))
            ps2 = psC.tile([P, NCH * C], F32, tag="ps2")
            for lo_c, hi_c in ((0, 512), (512, NCH * C)):
                for kc in range(T):
                    nc.tensor.matmul(out=ps2[:, lo_c:hi_c], lhsT=rowcov[kc][:],
                                     rhs=rhs2[kc][:, lo_c:hi_c],
                                     start=(kc == 0), stop=(kc == T - 1))

            s1m = sing.tile([P, C], F32, tag="s1m")
            nc.vector.tensor_scalar(out=s1m[:], in0=ps1[:], scalar1=1.0,
                                    scalar2=None, op0=OP.max)
            e1 = sing.tile([P, C], I32, tag="e1")
            nc.vector.tensor_scalar(out=e1[:], in0=s1m[:].bitcast(I32),
                                    scalar1=23, scalar2=None,
                                    op0=OP.logical_shift_right)
            nc.vector.tensor_scalar(out=e1[:], in0=e1[:], scalar1=127,
                                    scalar2=None, op0=OP.subtract)
            cst_i = sing.tile([P, C], I32, tag="cst_i")
            nc.vector.tensor_scalar(out=cst_i[:], in0=e1[:], scalar1=9363,
                                    scalar2=None, op0=OP.mult)
            nc.vector.tensor_scalar(out=cst_i[:], in0=cst_i[:], scalar1=16,
                                    scalar2=None, op0=OP.arith_shift_right)
            cst_f = sing.tile([P, C], F32, tag="cst_f")
            nc.vector.tensor_copy(cst_f[:], cst_i[:])

            # word mean table matmuls (PE-parallel with the DVE chain above)
            for wt in range(WT):
                ptab = psD.tile([P, 1024], F32, tag="ptab", name=f"ptab{wt}")
                for kc in range(T):
                    lhs = Opr[kc][:, wt * P:(wt + 1) * P]
                    nc.tensor.matmul(out=ptab[:, 0:512], lhsT=lhs,
                                     rhs=embext[kc][:, 0:512],
                                     start=(kc == 0), stop=(kc == T - 1))
                    nc.tensor.matmul(out=ptab[:, 512:D + 1], lhsT=lhs,
                                     rhs=embext[kc][:, 512:D + 1],
                                     start=(kc == 0), stop=(kc == T - 1))
                rec = sing.tile([P, 1], F32, tag="rec")
                nc.vector.tensor_scalar(out=rec[:], in0=ptab[:, D:D + 1],
                                        scalar1=1.0, scalar2=None, op0=OP.max)
                recr = sing.tile([P, 1], F32, tag="recr")
                nc.vector.reciprocal(recr[:], rec[:])
                tb = sing.tile([P, D], F16, tag=f"table{wt}")
                nc.scalar.mul(out=tb[:], in_=ptab[:, 0:D], mul=recr[:, 0:1])
                table16.append(tb)

            # msel[r, c] = ps2[r, cstar, c] via one-hot mask + k-reduce
            cmp8 = sing.tile([P, NCH * C], F32, tag="cmp8")
            nc.vector.tensor_tensor(
                cmp8[:].rearrange("p (a b) -> p a b", a=NCH),
                iota8f.unsqueeze(2).broadcast_to([P, NCH, C]),
                cst_f[:].unsqueeze(1).broadcast_to([P, NCH, C]),
                OP.is_equal)
            nc.vector.tensor_tensor(cmp8[:], cmp8[:], ps2[:], OP.mult)
            msel = sing.tile([P, C], F32, tag="msel")
            nc.vector.tensor_reduce(msel[:],
                                    cmp8[:].rearrange("p (a b) -> p b a", a=NCH),
                                    mybir.AxisListType.X, OP.add)

            mm = sing.tile([P, C], F32, tag="mm")
            nc.vector.tensor_scalar(out=mm[:], in0=msel[:], scalar1=1.0,
                                    scalar2=None, op0=OP.max)
            e2 = sing.tile([P, C], I32, tag="e2")
            nc.vector.tensor_scalar(out=e2[:], in0=mm[:].bitcast(I32),
                                    scalar1=23, scalar2=None,
                                    op0=OP.logical_shift_right)
            nc.vector.tensor_scalar(out=e2[:], in0=e2[:], scalar1=127,
                                    scalar2=None, op0=OP.subtract)
            lo = sing.tile([P, C], I32, tag="lo")
            nc.vector.tensor_scalar(out=lo[:], in0=e2[:], scalar1=1,
                                    scalar2=None, op0=OP.arith_shift_right)
            nc.vector.tensor_scalar(out=widx_i[:], in0=cst_i[:], scalar1=5,
                                    scalar2=None, op0=OP.logical_shift_left)
            nc.vector.tensor_tensor(widx_i[:], widx_i[:], lo[:], OP.add)
            nc.vector.tensor_copy(widx16[:], widx_i[:])

        # round-trip through DRAM to flatten + broadcast across partitions
        nc.sync.dma_start(out=widx_dram[:], in_=widx16[:])
        widx_flat = widx_dram[:].rearrange("p c -> (p c)")
        widx_g = []
        for g in range(NG):
            wg = sing.tile([P, PG], F16, tag=f"widx_g{g}")
            nc.gpsimd.dma_start(
                out=wg[:],
                in_=widx_flat[g * PG:(g + 1) * PG].partition_broadcast(P))
            widx_g.append(wg)

        # ---- paint: out[d, p] = table[widx[p], d] ----
        with tc.tile_pool(name="oh", bufs=2) as ohp, \
             tc.tile_pool(name="stage", bufs=4) as stp, \
             tc.tile_pool(name="pp", bufs=2, space="PSUM") as ppp:
            for g in range(NG):
                gs = slice(g * PG, (g + 1) * PG)
                ohs = []
                for kc in range(WT):
                    oh = ohp.tile([P, PG], F16, tag=f"oh{kc}", name=f"oh{kc}")
                    nc.vector.tensor_scalar(out=oh[:], in0=widx_g[g][:],
                                            scalar1=iotawp[:, kc:kc + 1],
                                            scalar2=None, op0=OP.is_equal)
                    ohs.append(oh)
                for dt in range(DT):
                    stage = stp.tile([P, PG], F32, tag="stage", name="stage")
                    pp = ppp.tile([P, PG], F32, tag="pp", name="pp")
                    dsl = slice(dt * P, (dt + 1) * P)
                    for kc in range(WT):
                        for s3 in range(NH):
                            nc.tensor.matmul(
                                out=pp[:, s3 * 512:(s3 + 1) * 512],
                                lhsT=table16[kc][:, dsl],
                                rhs=ohs[kc][:, s3 * 512:(s3 + 1) * 512],
                                start=(kc == 0), stop=(kc == WT - 1))
                    if dt % 2 == 0:
                        nc.scalar.copy(out=stage[:], in_=pp[:])
                        nc.sync.dma_start(out=out_ext[dt * P:(dt + 1) * P, gs],
                                          in_=stage[:])
                    else:
                        nc.vector.tensor_copy(stage[:], pp[:])
                        nc.scalar.dma_start(out=out_ext[dt * P:(dt + 1) * P, gs],
                                            in_=stage[:])
    nc.compile()
    return nc


_nc_cache = None


def kernel(bert_embeddings, coors, mask, image_h=1024, image_w=768, stride=8):
    global _last_results, _nc_cache
    emb = np.ascontiguousarray(np.asarray(bert_embeddings, dtype=np.float32))
    co = np.ascontiguousarray(np.asarray(coors, dtype=np.int32))
    mk = np.ascontiguousarray(np.asarray(mask, dtype=np.int32))
    ih, iw, st = int(image_h), int(image_w), int(stride)
    B = emb.shape[0]
    assert (ih // st, iw // st) == (R, C) and st == STRIDE
    assert emb.shape == (B, S, D) and B == 8

    if _nc_cache is None:
        _nc_cache = _build()
    nc = _nc_cache

    in_maps = [{"emb": emb[b], "coors": co[b], "mask": mk[b].reshape(S, 1)}
               for b in range(B)]
    res = run_bass_kernel_spmd(nc, in_maps, core_ids=list(range(B)))
    _last_results = res
    out = np.stack([np.asarray(res.results[b]["out"]).reshape(D, R, C)
                    for b in range(B)])
    return out.astype(np.float32)


# revision 7
# speedup vs baseline: 1.3093x; 1.0041x over previous
"""BERTgrid generator kernel for Trainium2 (8 NeuronCores, batch-parallel).

Per core (one document):
  emb [512, 768] f32, coors [512, 4] i32, mask [512, 1] i32
  -> out [768, 128*96] f32   (channel-major grid)

Device algorithm (no host compute on input values):
  1. valid/new_word/seg via triangular-matmul cumsums.
  2. Word mean table (shifted by one word) via one-hot matmul + reciprocal.
     Words span >=2 tokens, so word ids < 256 -> 2 chunks of 128.
  3. Per-pixel last-covering-word index via ONE exponent-weighted matmul:
     ps[r,c] = sum over covering new-word tokens of 2^(seg+1-127); distinct
     exponents per word make the f32 exponent of the sum exactly seg_max+1,
     so widx = (bits >> 23) - 1 (-1 where uncovered).
  4. Paint: out[d, p] = table[widx[p], d] as table^T @ onehot(widx) in fp16
     (one-hot has a single 1 per column, so fp16 only rounds table values).
"""

import sys

import numpy as np

try:
    import concourse.bass as bass
except ImportError:  # grading env fallback
    sys.path.insert(0, "/opt/trn_rl_repo")
    import concourse.bass as bass

from concourse import bacc
import concourse.tile as tile
from concourse import mybir
from concourse.bass_utils import run_bass_kernel_spmd
from contextlib import ExitStack

P = 128
S, D = 512, 768
R, C, STRIDE = 128, 96, 8
T = S // P            # token tiles
NW = 256              # max word ids (each word spans >=2 tokens)
WT = NW // P          # word chunks for table/paint (2)
NPIX = R * C          # 12288
PG = 2048             # pixels per paint group
NG = NPIX // PG
DT = D // P
NH = PG // 512        # matmul column-slices per psum tile

F32 = mybir.dt.float32
F16 = mybir.dt.float16
BF16 = mybir.dt.bfloat16
I32 = mybir.dt.int32
OP = mybir.AluOpType

_last_results = None


def _const_blocks():
    """Host-precomputed constants, embedded in the NEFF (input-independent)."""
    p = np.arange(P, dtype=np.float32)[:, None]
    iota_r = np.broadcast_to(np.arange(R, dtype=np.float32), (P, R))
    iota_c = np.broadcast_to(np.arange(C, dtype=np.float32), (P, C))
    iota_w = np.broadcast_to(np.arange(NW, dtype=np.float32) - 1.0, (P, NW))
    iota_wp = p + np.array([0.0, P], dtype=np.float32)[None, :]  # [P, WT]
    cf32 = np.concatenate([iota_r, iota_c, iota_w, iota_wp], axis=1)
    ii = np.arange(P)
    tri = (ii[:, None] <= ii[None, :]).astype(np.float16)  # [j, i] = j <= i
    ones = np.ones((P, P), dtype=np.float16)
    cf16 = np.concatenate([tri, ones], axis=1)
    return np.ascontiguousarray(cf32), np.ascontiguousarray(cf16)


def _build():
    nc = bacc.Bacc(None, target_bir_lowering=False)
    emb_ext = nc.declare_dram_parameter("emb", [S, D], F32, isOutput=False)
    coors_ext = nc.declare_dram_parameter("coors", [S, 4], I32, isOutput=False)
    mask_ext = nc.declare_dram_parameter("mask", [S, 1], I32, isOutput=False)
    out_ext = nc.declare_dram_parameter("out", [D, NPIX], F32, isOutput=True)
    widx_dram = nc.dram_tensor("widx_scratch", [P, C], F16)
    cf32_np, cf16_np = _const_blocks()
    cf32_ext = nc.inline_tensor(cf32_np, "cons_f32")
    cf16_ext = nc.inline_tensor(cf16_np, "cons_f16")

    with tile.TileContext(nc) as tc, ExitStack() as ctx:
        sing = ctx.enter_context(tc.tile_pool(name="sing", bufs=1))

        # ---- const loads (scalar HWDGE queue; sync is used by inputs) ----
        cf16 = sing.tile([P, 2 * P], F16, tag="cf16")
        nc.scalar.dma_start(out=cf16[:], in_=cf16_ext[:])
        NC32 = R + C + NW + WT
        cf32 = sing.tile([P, NC32], F32, tag="cf32")
        nc.scalar.dma_start(out=cf32[:], in_=cf32_ext[:])
        o_w, o_wp = R + C, R + C + NW
        iota_r = cf32[:, 0:R]
        iota_c = cf32[:, R:R + C]
        iotaW = cf32[:, o_w:o_w + NW]
        iotawp = cf32[:, o_wp:o_wp + WT]
        tri = cf16[:, 0:P]
        ones16 = cf16[:, P:2 * P]

        # ---- input loads (batched via rearranged APs) ----
        mask_all = sing.tile([P, T], I32, tag="mask_all")
        nc.sync.dma_start(
            out=mask_all[:].rearrange("p (t o) -> p t o", t=T),
            in_=mask_ext[:].rearrange("(t p) o -> p t o", t=T))
        coors_all = sing.tile([P, 4 * T], I32, tag="coors_all")
        coors_re = coors_ext[:].rearrange("(t p) c -> p t c", t=T)
        nc.sync.dma_start(
            out=coors_all[:].rearrange("p (t c) -> p t c", t=T),
            in_=coors_re)
        # prev-token coors: full-width loads only (1-partition DMAs are slow)
        coorsm1_all = sing.tile([P, 4 * T], I32, tag="coorsm1_all")
        nc.vector.memset(coorsm1_all[0:1, 0:4], -1)
        nc.sync.dma_start(
            out=coorsm1_all[1:P, 0:4].rearrange("p (t c) -> p t c", t=1),
            in_=coors_re[0:P - 1, 0:1])
        nc.sync.dma_start(
            out=coorsm1_all[:, 4:4 * T].rearrange("p (t c) -> p t c", t=T - 1),
            in_=coors_ext[P - 1:S - 1].rearrange("(t p) c -> p t c", t=T - 1))
        # emb as f16 with a trailing ones column per token tile (for counts)
        embT = sing.tile([P, T * (D + 1)], F16, tag="embT")
        embT3 = embT[:].rearrange("p (t e) -> p t e", e=D + 1)
        nc.vector.memset(embT3[:, :, D:D + 1], 1.0)
        nc.gpsimd.dma_start(
            out=embT3[:, :, 0:D],
            in_=emb_ext[:].rearrange("(t p) d -> p t d", t=T))
        embext = [embT[:, t * (D + 1):(t + 1) * (D + 1)] for t in range(T)]

        # ---- batched per-token quantities ----
        mf = sing.tile([P, T], F32, tag="maskf")
        nc.vector.tensor_copy(mf[:], mask_all[:])
        invm4 = sing.tile([P, T], F16, tag="invm4")
        nc.vector.tensor_scalar(out=invm4[:], in0=mf[:], scalar1=1.0,
                                scalar2=-1.0, op0=OP.subtract, op1=OP.mult)
        cf = sing.tile([P, 4 * T], F32, tag="coorsf")
        nc.vector.tensor_copy(cf[:], coors_all[:])
        cm1f = sing.tile([P, 4 * T], F32, tag="coorsm1f")
        nc.vector.tensor_copy(cm1f[:], coorsm1_all[:])
        eq16 = sing.tile([P, 4 * T], F32, tag="eq16")
        nc.vector.tensor_tensor(eq16[:], cf[:], cm1f[:], OP.is_equal)
        same4 = sing.tile([P, T], F32, tag="same4")
        nc.vector.tensor_reduce(same4[:],
                                eq16[:].rearrange("p (t c) -> p t c", t=T),
                                mybir.AxisListType.X, OP.min)
        wci = sing.tile([P, 4 * T], I32, tag="wci")
        nc.vector.tensor_scalar(out=wci[:], in0=coors_all[:], scalar1=3,
                                scalar2=None, op0=OP.arith_shift_right)
        wcf = sing.tile([P, 4 * T], F32, tag="wcf")
        nc.vector.tensor_copy(wcf[:], wci[:])

        # ---- valid = (cumsum(1-mask) == 0), seg = cumsum(new_word) - 1 ----
        valid4 = sing.tile([P, T], F32, tag="valid4")
        nw4 = sing.tile([P, T], F32, tag="nw4")
        nwb4 = sing.tile([P, T], F16, tag="nwb4")
        seg4 = sing.tile([P, T], F32, tag="seg4")
        segi4 = sing.tile([P, T], I32, tag="segi4")
        with tc.tile_pool(name="psA", bufs=1, space="PSUM") as psA:
            vps = psA.tile([P, T], F32, tag="vps", name="vps")
            for mt in range(T):
                for kc in range(mt + 1):
                    nc.tensor.matmul(out=vps[:, mt:mt + 1],
                                     lhsT=(tri if kc == mt else ones16),
                                     rhs=invm4[:, kc:kc + 1],
                                     start=(kc == 0), stop=(kc == mt))
            nc.vector.tensor_scalar(out=valid4[:], in0=vps[:],
                                    scalar1=0.5, scalar2=None, op0=OP.is_lt)
            nc.vector.scalar_tensor_tensor(out=nw4[:], in0=same4[:], scalar=0.5,
                                           in1=valid4[:], op0=OP.is_lt,
                                           op1=OP.mult)
            nc.vector.tensor_copy(nwb4[:], nw4[:])

            # coverage masks only need wcf -- keep DVE busy during cumsums
            rowcov, colcov = [], []
            for t in range(T):
                y0, y1 = wcf[:, 4 * t + 1:4 * t + 2], wcf[:, 4 * t + 3:4 * t + 4]
                x0, x1 = wcf[:, 4 * t + 0:4 * t + 1], wcf[:, 4 * t + 2:4 * t + 3]
                tge = sing.tile([P, R], F32, tag="tge")
                nc.vector.tensor_scalar(out=tge[:], in0=iota_r, scalar1=y0,
                                        scalar2=None, op0=OP.is_ge)
                rc = sing.tile([P, R], BF16, tag=f"rowcov{t}")
                nc.vector.scalar_tensor_tensor(out=rc[:], in0=iota_r, scalar=y1,
                                               in1=tge[:], op0=OP.is_lt,
                                               op1=OP.mult)
                rowcov.append(rc)
                cge = sing.tile([P, C], F32, tag="cge")
                nc.vector.tensor_scalar(out=cge[:], in0=iota_c, scalar1=x0,
                                        scalar2=None, op0=OP.is_ge)
                ccv = sing.tile([P, C], BF16, tag=f"colcov{t}")
                nc.vector.scalar_tensor_tensor(out=ccv[:], in0=iota_c, scalar=x1,
                                               in1=cge[:], op0=OP.is_lt,
                                               op1=OP.mult)
                colcov.append(ccv)

            sps = psA.tile([P, T], F32, tag="sps", name="sps")
            for mt in range(T):
                for kc in range(mt + 1):
                    nc.tensor.matmul(out=sps[:, mt:mt + 1],
                                     lhsT=(tri if kc == mt else ones16),
                                     rhs=nwb4[:, kc:kc + 1],
                                     start=(kc == 0), stop=(kc == mt))
            nc.vector.tensor_scalar(out=seg4[:], in0=sps[:],
                                    scalar1=1.0, scalar2=None, op0=OP.subtract)
            nc.vector.tensor_copy(segi4[:], seg4[:])

        # ---- single-stage scan weight: cw = 2^(seg+1-127) * new_word ----
        swb = sing.tile([P, T], I32, tag="swb")
        nc.vector.tensor_scalar(out=swb[:], in0=segi4[:], scalar1=1,
                                scalar2=None, op0=OP.add)
        nc.vector.tensor_scalar(out=swb[:], in0=swb[:], scalar1=23,
                                scalar2=None, op0=OP.logical_shift_left)
        cw = sing.tile([P, T], F32, tag="cw")
        nc.vector.tensor_tensor(cw[:], swb[:].bitcast(F32), nw4[:], OP.mult)
        rhs1 = []
        for t in range(T):
            r1 = sing.tile([P, C], BF16, tag=f"rhs1{t}")
            nc.vector.tensor_scalar(out=r1[:], in0=colcov[t][:],
                                    scalar1=cw[:, t:t + 1],
                                    scalar2=None, op0=OP.mult)
            rhs1.append(r1)

        # ---- index map: one matmul stage, widx = exponent - 1 ----
        widx16 = sing.tile([P, C], F16, tag="widx16")
        widx_i = sing.tile([P, C], I32, tag="widx_i")
        table16 = []
        with tc.tile_pool(name="psC", bufs=1, space="PSUM") as psC, \
             tc.tile_pool(name="psD", bufs=2, space="PSUM") as psD:
            ps1 = psC.tile([P, C], F32, tag="ps1")
            for kc in range(T):
                nc.tensor.matmul(out=ps1[:], lhsT=rowcov[kc][:], rhs=rhs1[kc][:],
                                 start=(kc == 0), stop=(kc == T - 1))
            nc.vector.tensor_scalar(out=widx_i[:], in0=ps1[:].bitcast(I32),
                                    scalar1=23, scalar2=None,
                                    op0=OP.logical_shift_right)
            nc.vector.tensor_scalar(out=widx_i[:], in0=widx_i[:], scalar1=1,
                                    scalar2=None, op0=OP.subtract)
            nc.vector.tensor_copy(widx16[:], widx_i[:])
            nc.sync.dma_start(out=widx_dram[:], in_=widx16[:])
            widx_flat = widx_dram[:].rearrange("p c -> (p c)")
            widx_g = []
            for g in range(NG):
                wg = sing.tile([P, PG], F16, tag=f"widx_g{g}")
                nc.gpsimd.dma_start(
                    out=wg[:],
                    in_=widx_flat[g * PG:(g + 1) * PG].partition_broadcast(P))
                widx_g.append(wg)

            # ---- word mean table (overlaps the widx DMA round-trip) ----
            # O'[i, w] = valid[i]*(seg[i] == w-1); table[w] = sum/cnt, row 0 = 0
            Opr = []
            for t in range(T):
                o = sing.tile([P, NW], F16, tag=f"op{t}")
                nc.vector.tensor_scalar(out=o[:], in0=iotaW,
                                        scalar1=seg4[:, t:t + 1],
                                        scalar2=valid4[:, t:t + 1],
                                        op0=OP.is_equal, op1=OP.mult)
                Opr.append(o)
            for wt in range(WT):
                ptab = psD.tile([P, 1024], F32, tag="ptab", name=f"ptab{wt}")
                for kc in range(T):
                    lhs = Opr[kc][:, wt * P:(wt + 1) * P]
                    nc.tensor.matmul(out=ptab[:, 0:512], lhsT=lhs,
                                     rhs=embext[kc][:, 0:512],
                                     start=(kc == 0), stop=(kc == T - 1))
                    nc.tensor.matmul(out=ptab[:, 512:D + 1], lhsT=lhs,
                                     rhs=embext[kc][:, 512:D + 1],
                                     start=(kc == 0), stop=(kc == T - 1))
                rec = sing.tile([P, 1], F32, tag="rec")
                nc.vector.tensor_scalar(out=rec[:], in0=ptab[:, D:D + 1],
                                        scalar1=1.0, scalar2=None, op0=OP.max)
                recr = sing.tile([P, 1], F32, tag="recr")
                nc.vector.reciprocal(recr[:], rec[:])
                tb = sing.tile([P, D], F16, tag=f"table{wt}")
                nc.scalar.mul(out=tb[:], in_=ptab[:, 0:D], mul=recr[:, 0:1])
                table16.append(tb)

        # ---- paint: out[d, p] = table[widx[p], d] ----
        with tc.tile_pool(name="oh", bufs=2) as ohp, \
             tc.tile_pool(name="stage", bufs=4) as stp, \
             tc.tile_pool(name="pp", bufs=2, space="PSUM") as ppp:
            for g in range(NG):
                gs = slice(g * PG, (g + 1) * PG)
                ohs = []
                for kc in range(WT):
                    oh = ohp.tile([P, PG], F16, tag=f"oh{kc}", name=f"oh{kc}")
                    nc.vector.tensor_scalar(out=oh[:], in0=widx_g[g][:],
                                            scalar1=iotawp[:, kc:kc + 1],
                                            scalar2=None, op0=OP.is_equal)
                    ohs.append(oh)
                for dt in range(DT):
                    stage = stp.tile([P, PG], F32, tag="stage", name="stage")
                    pp = ppp.tile([P, PG], F32, tag="pp", name="pp")
                    dsl = slice(dt * P, (dt + 1) * P)
                    for kc in range(WT):
                        for s3 in range(NH):
                            nc.tensor.matmul(
                                out=pp[:, s3 * 512:(s3 + 1) * 512],
                                lhsT=table16[kc][:, dsl],
                                rhs=ohs[kc][:, s3 * 512:(s3 + 1) * 512],
                                start=(kc == 0), stop=(kc == WT - 1))
                    if dt % 2 == 0:
                        nc.scalar.copy(out=stage[:], in_=pp[:])
                        nc.sync.dma_start(out=out_ext[dt * P:(dt + 1) * P, gs],
                                          in_=stage[:])
                    else:
                        nc.vector.tensor_copy(stage[:], pp[:])
                        nc.scalar.dma_start(out=out_ext[dt * P:(dt + 1) * P, gs],
                                            in_=stage[:])
    nc.compile()
    return nc


_nc_cache = None


def kernel(bert_embeddings, coors, mask, image_h=1024, image_w=768, stride=8):
    global _last_results, _nc_cache
    emb = np.ascontiguousarray(np.asarray(bert_embeddings, dtype=np.float32))
    co = np.ascontiguousarray(np.asarray(coors, dtype=np.int32))
    mk = np.ascontiguousarray(np.asarray(mask, dtype=np.int32))
    ih, iw, st = int(image_h), int(image_w), int(stride)
    B = emb.shape[0]
    assert (ih // st, iw // st) == (R, C) and st == STRIDE
    assert emb.shape == (B, S, D) and B == 8

    if _nc_cache is None:
        _nc_cache = _build()
    nc = _nc_cache

    in_maps = [{"emb": emb[b], "coors": co[b], "mask": mk[b].reshape(S, 1)}
               for b in range(B)]
    res = run_bass_kernel_spmd(nc, in_maps, core_ids=list(range(B)))
    _last_results = res
    out = np.stack([np.asarray(res.results[b]["out"]).reshape(D, R, C)
                    for b in range(B)])
    return out.astype(np.float32)
